# revision 7
# baseline (speedup 1.0000x reference)
"""Trainium2 Bass kernel for causal GQA self-attention (B=2, T=2048, C=2048,
Hq=16, Hkv=4, d=128, RoPE base 1e6).

Sharding: 8 cores = 2 batches x 4 kv-head groups. Each core computes, for its
(batch b, kv group g): the q/k/v projections restricted to that group (4 q
heads + 1 kv head), RoPE, causal attention, and the partial o_proj
(y_group @ Wo[group rows]). The host sums the 4 partial o_proj outputs per
batch (the all-reduce/unshard step of tensor parallelism).

Device schedule (v2):
  - Phase 1 projections run ci-outer / t4-inner with per-t4 PSUM banks so a
    weight chunk is stationary across 4 back-to-back matmuls, and k+v are
    interleaved per-ci so PE consumption (~1.7us/ci) matches DMA arrival
    (~1.6us/ci) from the first microsecond.
  - RoPE runs fully in bf16 (cos/sin tables shipped bf16) for 2x DVE rate.
  - Attention is software-pipelined: scores for step g+1 issue before the
    AV/rowsum matmuls of step g, so the ACT exp latency hides entirely under
    PE work. Scores are computed transposed (S^T = k @ qT) so exp needs no
    cross-partition work.
  - softmax 1/rowsum: rowsums accumulate via a ones-vector matmul in PSUM;
    the reciprocal runs as one custom-DVE op straight out of PSUM, is
    broadcast across partitions by the (otherwise idle) GpSimd engine, and
    multiplies yT straight out of its PSUM bank -- no DRAM round trip, no
    ACT log/exp chain.
  - o_proj work is queued and interleaved one (ti,nj) group per attention
    step (covers per-head normalization chains); qc runs ascending so only
    the last qc's o_proj groups remain for the tail, processed with a 4-bank
    PSUM pool.
  - the v bias is folded out entirely: since softmax rows sum to 1, bv
    contributes the constant row bv_tiled @ Wo_g, added on the host.
"""

import numpy as np
import ml_dtypes

import concourse.bass as bass
import concourse.mybir as mybir
from concourse import bacc
from concourse.tile import TileContext
from concourse.bass_utils import run_bass_kernel_spmd
from concourse.masks import make_identity

BF16 = mybir.dt.bfloat16
F32 = mybir.dt.float32

T = 2048
C = 2048
D = 128
NH = 4           # q heads per core
CI = C // 128    # contraction chunks
TC = T // 512    # t chunks of 512
TB = T // 128    # t blocks of 128
SCALE = 1.0 / np.sqrt(D)

_PROGRAM = None


def _ts(i, s):
    return bass.ts(i, s)


def _patch_act_tables():
    """Force every ACT function this kernel uses to resolve to the
    natural_log_exp_and_others table set, so the whole kernel needs exactly
    one ACT_TABLE_LOAD. Returns an undo callable."""
    import concourse.bacc as bacc_mod

    orig = bacc_mod.get_activation_tables
    A = mybir.ActivationFunctionType
    mine = {A.Exp, A.Ln, A.Identity, A.Copy}

    def patched(arch):
        tables = dict(orig(arch))
        for name in tables:
            if name != "natural_log_exp_and_others":
                tables[name] = set(tables[name]) - mine
        return tables

    bacc_mod.get_activation_tables = patched

    def undo():
        bacc_mod.get_activation_tables = orig

    return undo


def _build_program():
    undo = _patch_act_tables()
    try:
        return _build_program_inner()
    finally:
        undo()


def _build_program_inner():
    nc = bacc.Bacc("TRN2", target_bir_lowering=False, debug=False, num_devices=8)

    xT_d = nc.dram_tensor("xT", [C, T], BF16, kind="ExternalInput").ap()
    wq_d = nc.dram_tensor("wq", [C, NH * D], BF16, kind="ExternalInput").ap()
    wk_d = nc.dram_tensor("wk", [C, D], BF16, kind="ExternalInput").ap()
    wv_d = nc.dram_tensor("wv", [C, D], BF16, kind="ExternalInput").ap()
    wo_d = nc.dram_tensor("wo", [NH * D, C], BF16, kind="ExternalInput").ap()
    bq_d = nc.dram_tensor("bq", [D, NH], F32, kind="ExternalInput").ap()
    bk_d = nc.dram_tensor("bk", [D, 1], F32, kind="ExternalInput").ap()
    cos_d = nc.dram_tensor("cosT", [D, T], BF16, kind="ExternalInput").ap()
    sin_d = nc.dram_tensor("sinT", [D, T], BF16, kind="ExternalInput").ap()
    tri_d = nc.dram_tensor("tri", [D, D], BF16, kind="ExternalInput").ap()
    out_d = nc.dram_tensor("out", [T, C], F32, kind="ExternalOutput").ap()

    Ident = mybir.ActivationFunctionType.Identity
    Exp = mybir.ActivationFunctionType.Exp

    with TileContext(nc) as tc:
        with (
            tc.tile_pool(name="consts", bufs=1) as consts,
            tc.tile_pool(name="acts", bufs=1) as acts,
        ):
            # ---- resident constants, DMAs issued in consumption order ----
            xT_sb = consts.tile([128, CI, T], BF16)
            wq_sb = consts.tile([128, CI, NH * D], BF16)
            wk_sb = consts.tile([128, CI, D], BF16)
            wv_sb = consts.tile([128, CI, D], BF16)
            wo_sb = consts.tile([128, NH, C], BF16)
            bq_sb = consts.tile([128, NH], F32)
            bk_sb = consts.tile([128, 1], F32)
            cos_sb = consts.tile([128, T], BF16)
            sin_sb = consts.tile([128, T], BF16)
            tri_sb = consts.tile([128, 128], BF16)
            ones_sb = consts.tile([128, 1], BF16)
            ident_sb = consts.tile([128, 128], BF16)

            nc.sync.dma_start(out=bk_sb[:], in_=bk_d[:])
            nc.sync.dma_start(out=bq_sb[:], in_=bq_d[:])
            for ci in range(CI):
                nc.sync.dma_start(out=wk_sb[:, ci, :], in_=wk_d[_ts(ci, 128), :])
                nc.sync.dma_start(
                    out=wq_sb[:, ci, _ts(0, 128)], in_=wq_d[_ts(ci, 128), _ts(0, 128)]
                )
                nc.sync.dma_start(out=xT_sb[:, ci, :], in_=xT_d[_ts(ci, 128), :])
            for ci in range(CI):
                nc.sync.dma_start(
                    out=wq_sb[:, ci, _ts(1, 128)], in_=wq_d[_ts(ci, 128), _ts(1, 128)]
                )
            nc.sync.dma_start(out=cos_sb[:], in_=cos_d[:])
            nc.sync.dma_start(out=sin_sb[:], in_=sin_d[:])
            for h in (2, 3):
                for ci in range(CI):
                    nc.sync.dma_start(
                        out=wq_sb[:, ci, _ts(h, 128)],
                        in_=wq_d[_ts(ci, 128), _ts(h, 128)],
                    )
            for ci in range(CI):
                nc.sync.dma_start(out=wv_sb[:, ci, :], in_=wv_d[_ts(ci, 128), :])
            nc.sync.dma_start(out=tri_sb[:], in_=tri_d[:])
            for h in range(NH):
                nc.sync.dma_start(out=wo_sb[:, h, :], in_=wo_d[_ts(h, 128), :])
            nc.vector.memset(ones_sb[:], 1.0)
            make_identity(nc, ident_sb[:])

            # ---- persistent activations ---------------------------------
            qT_all = acts.tile([128, NH, T], BF16)   # rotated q^T per head
            kT_all = acts.tile([128, T], BF16)       # rotated k^T
            v_sb = acts.tile([128, TB, D], BF16)     # v in natural [t, d] blocks
            yTn = acts.tile([128, NH, T], BF16)      # normalized y^T per head

            def rope_chunk(ps, t4, bias_ap, dest, pool):
                # psum -> bf16 with bias, then rotate-half (partition swap via
                # SB->SB DMA; sign baked into the sin table) + cos/sin muls
                qb = pool.tile([128, 512], BF16, tag="qb")
                nc.scalar.activation(qb[:], ps[:], Ident, bias=bias_ap)
                sh = pool.tile([128, 512], BF16, tag="sh")
                nc.sync.dma_start(out=sh[0:64, :], in_=qb[64:128, :])
                nc.sync.dma_start(out=sh[64:128, :], in_=qb[0:64, :])
                t1 = pool.tile([128, 512], BF16, tag="t1")
                nc.vector.tensor_mul(t1[:], qb[:], cos_sb[:, _ts(t4, 512)])
                nc.vector.tensor_mul(sh[:], sh[:], sin_sb[:, _ts(t4, 512)])
                nc.vector.tensor_add(dest, t1[:], sh[:])

            # ---- phase 1: projections ----------------------------------
            # k+q0 interleaved per-ci (PE ~1.7us/ci matches DMA ~1.6us/ci),
            # then q1..q3 from resident x, v LAST so the phase tail is short
            # (DVE copy + PE transposes) instead of a full rope chain.
            with tc.tile_pool(name="rope", bufs=4) as rope_pool:
                with (
                    tc.tile_pool(name="kp", bufs=1, space="PSUM") as kp,
                    tc.tile_pool(name="q0p", bufs=1, space="PSUM") as q0p,
                ):
                    k_ps = [kp.tile([128, 512], F32, name=f"kps{t4}", tag=f"k{t4}") for t4 in range(TC)]
                    q0_ps = [q0p.tile([128, 512], F32, name=f"q0ps{t4}", tag=f"q0{t4}") for t4 in range(TC)]
                    for ci in range(CI):
                        for t4 in range(TC):
                            nc.tensor.matmul(
                                k_ps[t4][:],
                                wk_sb[:, ci, :],
                                xT_sb[:, ci, _ts(t4, 512)],
                                start=(ci == 0),
                                stop=(ci == CI - 1),
                            )
                        for t4 in range(TC):
                            nc.tensor.matmul(
                                q0_ps[t4][:],
                                wq_sb[:, ci, _ts(0, 128)],
                                xT_sb[:, ci, _ts(t4, 512)],
                                start=(ci == 0),
                                stop=(ci == CI - 1),
                            )
                    for t4 in range(TC):
                        rope_chunk(
                            k_ps[t4], t4, bk_sb[:, 0:1],
                            kT_all[:, _ts(t4, 512)], rope_pool,
                        )
                    for t4 in range(TC):
                        rope_chunk(
                            q0_ps[t4], t4, bq_sb[:, 0:1],
                            qT_all[:, 0, _ts(t4, 512)], rope_pool,
                        )

                # ---- q1..q3 projections + rope --------------------------
                with tc.tile_pool(name="qp", bufs=2, space="PSUM") as qp:
                    for h in range(1, NH):
                        q_ps = [qp.tile([128, 512], F32, name=f"qps{t4}", tag=f"q{t4}") for t4 in range(TC)]
                        for ci in range(CI):
                            for t4 in range(TC):
                                nc.tensor.matmul(
                                    q_ps[t4][:],
                                    wq_sb[:, ci, _ts(h, 128)],
                                    xT_sb[:, ci, _ts(t4, 512)],
                                    start=(ci == 0),
                                    stop=(ci == CI - 1),
                                )
                        for t4 in range(TC):
                            rope_chunk(
                                q_ps[t4], t4, bq_sb[:, h : h + 1],
                                qT_all[:, h, _ts(t4, 512)], rope_pool,
                            )

                # ---- v projection last: short tail into attention -------
                with (
                    tc.tile_pool(name="vp", bufs=1, space="PSUM") as vp,
                    tc.tile_pool(name="vtp", bufs=2, space="PSUM") as vtp,
                ):
                    v_ps = [vp.tile([128, 512], F32, name=f"vps{t4}", tag=f"v{t4}") for t4 in range(TC)]
                    for ci in range(CI):
                        for t4 in range(TC):
                            nc.tensor.matmul(
                                v_ps[t4][:],
                                wv_sb[:, ci, :],
                                xT_sb[:, ci, _ts(t4, 512)],
                                start=(ci == 0),
                                stop=(ci == CI - 1),
                            )
                    for t4 in range(TC):
                        vb = rope_pool.tile([128, 512], BF16, tag=f"vb")
                        nc.vector.tensor_copy(vb[:], v_ps[t4][:])
                        for j in range(4):
                            tb = t4 * 4 + j
                            pt = vtp.tile([128, 128], BF16)
                            nc.tensor.transpose(
                                pt[:], vb[:, _ts(j, 128)], ident_sb[:]
                            )
                            nc.vector.tensor_copy(v_sb[:, tb, :], pt[:])

            # ---- phase 2: attention, pipelined, with o_proj interleave --
            oproj_q = []

            def oproj_group(ti, nj, psum_pool, oe_pool, evict):
                ps = psum_pool.tile([128, 512], F32)
                for h2 in range(NH):
                    nc.tensor.matmul(
                        ps[:],
                        yTn[:, h2, _ts(ti, 128)],
                        wo_sb[:, h2, _ts(nj, 512)],
                        start=(h2 == 0),
                        stop=(h2 == NH - 1),
                    )
                oe = oe_pool.tile([128, 512], F32)
                if evict == "v":
                    nc.vector.tensor_copy(oe[:], ps[:])
                else:
                    nc.scalar.copy(oe[:], ps[:])
                nc.sync.dma_start(
                    out=out_d[_ts(ti, 128), _ts(nj, 512)], in_=oe[:]
                )

            with (
                tc.tile_pool(name="st", bufs=2, space="PSUM") as stp,
                tc.tile_pool(name="yt", bufs=2, space="PSUM") as ytp,
                tc.tile_pool(name="rs", bufs=1, space="PSUM") as rsp,
                tc.tile_pool(name="poi", bufs=1, space="PSUM") as poi,
                tc.tile_pool(name="ptp", bufs=3) as ptp,
                tc.tile_pool(name="rsb", bufs=2) as rsb,
                tc.tile_pool(name="bcb", bufs=2) as bcb,
                tc.tile_pool(name="oei", bufs=3) as oei,
            ):
                def scores_step(h, qc, g):
                    # scores for kb pair (2g, 2g+1) + exp
                    st = stp.tile([128, 1024], F32)
                    for u in range(2):
                        nc.tensor.matmul(
                            st[:, _ts(u, 512)],
                            kT_all[:, _ts(2 * g + u, 128)],
                            qT_all[:, h, _ts(qc, 512)],
                            start=True,
                            stop=True,
                        )
                    pt = ptp.tile([128, 1024], BF16)
                    nc.scalar.activation(pt[:], st[:], Exp, scale=SCALE)
                    return pt

                for qc in range(TC):  # ascending: smallest o_proj tail
                    nkb = 4 * (qc + 1)
                    G = nkb // 2
                    for h in range(NH):
                        yt_ps = ytp.tile([128, 512], F32)
                        rs_ps = rsp.tile([1, 512], F32)
                        pt = scores_step(h, qc, 0)
                        for g in range(G):
                            # pipeline: next step's scores + exp issue first,
                            # so PE has work while ACT computes exp(g)
                            pt_next = (
                                scores_step(h, qc, g + 1) if g + 1 < G else None
                            )
                            # causal mask on diagonal blocks
                            for u in range(2):
                                kb = 2 * g + u
                                j = kb - 4 * qc
                                if j >= 0:
                                    base = u * 512
                                    if j > 0:
                                        nc.vector.memset(
                                            pt[:, base : base + j * 128], 0.0
                                        )
                                    blk = pt[:, base + j * 128 : base + (j + 1) * 128]
                                    nc.vector.tensor_mul(blk, blk, tri_sb[:])
                            for u in range(2):
                                kb = 2 * g + u
                                nc.tensor.matmul(
                                    yt_ps[:],
                                    v_sb[:, kb, :],
                                    pt[:, _ts(u, 512)],
                                    start=(kb == 0),
                                    stop=(kb == nkb - 1),
                                )
                            for u in range(2):
                                kb = 2 * g + u
                                nc.tensor.matmul(
                                    rs_ps[:],
                                    ones_sb[:],
                                    pt[:, _ts(u, 512)],
                                    start=(kb == 0),
                                    stop=(kb == nkb - 1),
                                )
                            if oproj_q:
                                oproj_group(*oproj_q.pop(0), poi, oei, "s")
                            pt = pt_next
                        # head tail: 1/rowsum straight off PSUM (custom DVE),
                        # partition-broadcast on GpSimd, normalize out of PSUM
                        rsv = rsb.tile([1, 512], F32)
                        nc.vector.reciprocal_approx_fast(rsv[:], rs_ps[:])
                        bc = bcb.tile([128, 512], F32)
                        nc.gpsimd.partition_broadcast(bc[:], rsv[:], channels=128)
                        nc.vector.tensor_mul(
                            yTn[:, h, _ts(qc, 512)], yt_ps[:], bc[:]
                        )
                    oproj_q.extend(
                        (ti, nj)
                        for ti in range(4 * qc, 4 * qc + 4)
                        for nj in range(TC)
                    )

            # ---- tail: drain remaining o_proj with a wide PSUM pool -----
            with (
                tc.tile_pool(name="pot", bufs=4, space="PSUM") as pot,
                tc.tile_pool(name="oet", bufs=6) as oet,
            ):
                for i, (ti, nj) in enumerate(oproj_q):
                    oproj_group(ti, nj, pot, oet, "v" if i % 2 == 0 else "s")

    nc.finalize()
    return nc


def _get_program():
    global _PROGRAM
    if _PROGRAM is None:
        _PROGRAM = _build_program()
    return _PROGRAM


def _rope_tables():
    inv_freq = 1.0 / (1000000.0 ** (np.arange(0, D, 2, dtype=np.float64) / D))
    pos = np.arange(T, dtype=np.float64)
    si = np.outer(pos, inv_freq)                      # [T, D/2]
    cos_h, sin_h = np.cos(si), np.sin(si)
    cos = np.stack([cos_h, cos_h], axis=-1).reshape(T, D)
    sin = np.stack([sin_h, sin_h], axis=-1).reshape(T, D)
    cosT = np.ascontiguousarray(cos.T).astype(np.float32)   # [D, T]
    sinT = np.ascontiguousarray(sin.T).astype(np.float32)
    # rotate-half as a partition shift: sh[i<64]=q[i+64], sh[i>=64]=q[i-64];
    # q_rot = q*cos + sh*sin_signed with the -1 for i<64 baked into the table
    sinT[: D // 2] *= -1.0
    return cosT, sinT


def make_in_maps(x, Wq, bq, Wk, bk, Wv, bv, Wo):
    bf = ml_dtypes.bfloat16
    cosT, sinT = _rope_tables()
    tri = np.triu(np.ones((D, D), dtype=np.float32)).astype(bf)  # [k, q]: q >= k
    in_maps = []
    for b in range(2):
        xT = np.ascontiguousarray(x[b].T).astype(bf)
        for g in range(4):
            in_maps.append(
                {
                    "xT": xT,
                    "wq": np.ascontiguousarray(Wq[:, g * 512 : (g + 1) * 512]).astype(bf),
                    "wk": np.ascontiguousarray(Wk[:, g * 128 : (g + 1) * 128]).astype(bf),
                    "wv": np.ascontiguousarray(Wv[:, g * 128 : (g + 1) * 128]).astype(bf),
                    "wo": np.ascontiguousarray(Wo[g * 512 : (g + 1) * 512, :]).astype(bf),
                    "bq": np.ascontiguousarray(
                        bq[g * 512 : (g + 1) * 512].reshape(NH, D).T
                    ).astype(np.float32),
                    "bk": np.ascontiguousarray(
                        bk[g * 128 : (g + 1) * 128].reshape(D, 1)
                    ).astype(np.float32),
                    "cosT": cosT.astype(bf),
                    "sinT": sinT.astype(bf),
                    "tri": tri,
                }
            )
    return in_maps


def combine_outputs(res, inputs):
    bv, Wo = np.asarray(inputs["bv"]), np.asarray(inputs["Wo"])
    out = np.zeros((2, T, C), dtype=np.float32)
    for c in range(8):
        g = c % 4
        out[c // 4] += res.results[c]["out"]
        # v-bias contribution: softmax rows sum to 1, so bv adds the constant
        # row (bv tiled over the 4 q heads) @ Wo_group to every output row
        bv_tiled = np.tile(bv[g * 128 : (g + 1) * 128], NH).astype(np.float64)
        cvec = bv_tiled @ Wo[g * 512 : (g + 1) * 512, :].astype(np.float64)
        out[c // 4] += cvec.astype(np.float32)[None, :]
    return out


def kernel(x, Wq, bq, Wk, bk, Wv, bv, Wo):
    nc = _get_program()
    in_maps = make_in_maps(x, Wq, bq, Wk, bk, Wv, bv, Wo)
    res = run_bass_kernel_spmd(nc, in_maps, list(range(8)))
    return combine_outputs(res, {"bv": bv, "Wo": Wo})


# revision 9
# speedup vs baseline: 1.1281x; 1.1281x over previous
"""Trainium2 Bass kernel for causal GQA self-attention (B=2, T=2048, C=2048,
Hq=16, Hkv=4, d=128, RoPE base 1e6).

Sharding: 8 cores = 2 batches x 4 kv-head groups. Each core computes, for its
(batch b, kv group g): the q/k/v projections restricted to that group (4 q
heads + 1 kv head), RoPE, causal attention, and the partial o_proj
(y_group @ Wo[group rows]). The host sums the 4 partial o_proj outputs per
batch (the all-reduce/unshard step of tensor parallelism).

Device schedule (v2):
  - Phase 1 projections run ci-outer / t4-inner with per-t4 PSUM banks so a
    weight chunk is stationary across 4 back-to-back matmuls, and k+v are
    interleaved per-ci so PE consumption (~1.7us/ci) matches DMA arrival
    (~1.6us/ci) from the first microsecond.
  - RoPE runs fully in bf16 (cos/sin tables shipped bf16) for 2x DVE rate.
  - Attention is software-pipelined: scores for step g+1 issue before the
    AV/rowsum matmuls of step g, so the ACT exp latency hides entirely under
    PE work. Scores are computed transposed (S^T = k @ qT) so exp needs no
    cross-partition work.
  - softmax 1/rowsum: rowsums accumulate via a ones-vector matmul in PSUM;
    the reciprocal runs as one custom-DVE op straight out of PSUM, is
    broadcast across partitions by the (otherwise idle) GpSimd engine, and
    multiplies yT straight out of its PSUM bank -- no DRAM round trip, no
    ACT log/exp chain.
  - o_proj work is queued and interleaved one (ti,nj) group per attention
    step (covers per-head normalization chains); qc runs ascending so only
    the last qc's o_proj groups remain for the tail, processed with a 4-bank
    PSUM pool.
  - the v bias is folded out entirely: since softmax rows sum to 1, bv
    contributes the constant row bv_tiled @ Wo_g, added on the host.
"""

import numpy as np
import ml_dtypes

import concourse.bass as bass
import concourse.mybir as mybir
from concourse import bacc
from concourse.tile import TileContext
from concourse.bass_utils import run_bass_kernel_spmd
from concourse.masks import make_identity

BF16 = mybir.dt.bfloat16
F32 = mybir.dt.float32

T = 2048
C = 2048
D = 128
NH = 4           # q heads per core
CI = C // 128    # contraction chunks
TC = T // 512    # t chunks of 512
TB = T // 128    # t blocks of 128
SCALE = 1.0 / np.sqrt(D)

_PROGRAM = None


def _ts(i, s):
    return bass.ts(i, s)


def _patch_act_tables():
    """Force every ACT function this kernel uses to resolve to the
    natural_log_exp_and_others table set, so the whole kernel needs exactly
    one ACT_TABLE_LOAD. Returns an undo callable."""
    import concourse.bacc as bacc_mod

    orig = bacc_mod.get_activation_tables
    A = mybir.ActivationFunctionType
    mine = {A.Exp, A.Ln, A.Identity, A.Copy}

    def patched(arch):
        tables = dict(orig(arch))
        for name in tables:
            if name != "natural_log_exp_and_others":
                tables[name] = set(tables[name]) - mine
        return tables

    bacc_mod.get_activation_tables = patched

    def undo():
        bacc_mod.get_activation_tables = orig

    return undo


def _build_program():
    undo = _patch_act_tables()
    try:
        return _build_program_inner()
    finally:
        undo()


def _build_program_inner():
    nc = bacc.Bacc("TRN2", target_bir_lowering=False, debug=False, num_devices=8)

    xT_d = nc.dram_tensor("xT", [C, T], BF16, kind="ExternalInput").ap()
    wq_d = nc.dram_tensor("wq", [C, NH * D], BF16, kind="ExternalInput").ap()
    wkv_d = nc.dram_tensor("wkv", [C, 2 * D], BF16, kind="ExternalInput").ap()
    wo_d = nc.dram_tensor("wo", [NH * D, C], BF16, kind="ExternalInput").ap()
    bq_d = nc.dram_tensor("bq", [D, NH], F32, kind="ExternalInput").ap()
    bk_d = nc.dram_tensor("bk", [D, 1], F32, kind="ExternalInput").ap()
    cos_d = nc.dram_tensor("cosT", [D, T], BF16, kind="ExternalInput").ap()
    sin_d = nc.dram_tensor("sinT", [D, T], BF16, kind="ExternalInput").ap()
    tri_d = nc.dram_tensor("tri", [D, D], BF16, kind="ExternalInput").ap()
    out_d = nc.dram_tensor("out", [T, C], F32, kind="ExternalOutput").ap()

    Ident = mybir.ActivationFunctionType.Identity
    Exp = mybir.ActivationFunctionType.Exp

    with TileContext(nc) as tc:
        with (
            tc.tile_pool(name="consts", bufs=1) as consts,
            tc.tile_pool(name="acts", bufs=1) as acts,
        ):
            # ---- resident constants, DMAs issued in consumption order ----
            xT_sb = consts.tile([128, CI, T], BF16)
            wq_sb = consts.tile([128, CI, NH * D], BF16)
            wkv_sb = consts.tile([128, CI, 2 * D], BF16)
            wo_sb = consts.tile([128, NH, C], BF16)
            bq_sb = consts.tile([128, NH], F32)
            bk_sb = consts.tile([128, 1], F32)
            cos_sb = consts.tile([128, T], BF16)
            sin_sb = consts.tile([128, T], BF16)
            tri_sb = consts.tile([128, 128], BF16)
            ones_sb = consts.tile([128, 1], BF16)
            ident_sb = consts.tile([128, 128], BF16)

            nc.sync.dma_start(out=bk_sb[:], in_=bk_d[:])
            nc.sync.dma_start(out=bq_sb[:], in_=bq_d[:])
            for ci in range(CI):
                nc.sync.dma_start(out=wkv_sb[:, ci, :], in_=wkv_d[_ts(ci, 128), :])
                nc.sync.dma_start(out=xT_sb[:, ci, :], in_=xT_d[_ts(ci, 128), :])
            for ci in range(CI):
                nc.sync.dma_start(out=wq_sb[:, ci, :], in_=wq_d[_ts(ci, 128), :])
            nc.sync.dma_start(out=cos_sb[:], in_=cos_d[:])
            nc.sync.dma_start(out=sin_sb[:], in_=sin_d[:])
            nc.sync.dma_start(out=tri_sb[:], in_=tri_d[:])
            for h in range(NH):
                nc.sync.dma_start(out=wo_sb[:, h, :], in_=wo_d[_ts(h, 128), :])
            nc.vector.memset(ones_sb[:], 1.0)
            make_identity(nc, ident_sb[:])

            # ---- persistent activations ---------------------------------
            qT_all = acts.tile([128, NH, T], BF16)   # rotated q^T per head
            kT_all = acts.tile([128, T], BF16)       # rotated k^T
            v_sb = acts.tile([128, TB, D], BF16)     # v in natural [t, d] blocks
            yTn = acts.tile([128, NH, T], BF16)      # normalized y^T per head

            def rope_chunk(ps, t4, bias_ap, dest, pool):
                # psum -> bf16 with bias; rotate-half done as two ACT
                # half-evictions straight from PSUM (partition swap, sign
                # baked into the sin table), then cos/sin muls on DVE
                qb = pool.tile([128, 512], BF16, tag="qb")
                nc.scalar.activation(qb[:], ps[:], Ident, bias=bias_ap)
                sh = pool.tile([128, 512], BF16, tag="sh")
                nc.scalar.activation(
                    sh[0:64, :], ps[64:128, :], Ident, bias=bias_ap[64:128, :]
                )
                nc.scalar.activation(
                    sh[64:128, :], ps[0:64, :], Ident, bias=bias_ap[0:64, :]
                )
                t1 = pool.tile([128, 512], BF16, tag="t1")
                nc.vector.tensor_mul(t1[:], qb[:], cos_sb[:, _ts(t4, 512)])
                nc.vector.tensor_mul(sh[:], sh[:], sin_sb[:, _ts(t4, 512)])
                nc.vector.tensor_add(dest, t1[:], sh[:])

            # ---- phase 1: projections ----------------------------------
            # k+v interleaved per-ci (8 PSUM banks; PE ~1.7us/ci matches the
            # 2-dispatch DMA stream), then v transposes, then q0..q3.
            with tc.tile_pool(name="rope", bufs=4) as rope_pool:
                with (
                    tc.tile_pool(name="kp", bufs=1, space="PSUM") as kp,
                    tc.tile_pool(name="vp", bufs=1, space="PSUM") as vp,
                ):
                    k_ps = [kp.tile([128, 512], F32, name=f"kps{t4}", tag=f"k{t4}") for t4 in range(TC)]
                    v_ps = [vp.tile([128, 512], F32, name=f"vps{t4}", tag=f"v{t4}") for t4 in range(TC)]
                    for ci in range(CI):
                        for t4 in range(TC):
                            nc.tensor.matmul(
                                k_ps[t4][:],
                                wkv_sb[:, ci, 0:128],
                                xT_sb[:, ci, _ts(t4, 512)],
                                start=(ci == 0),
                                stop=(ci == CI - 1),
                            )
                        for t4 in range(TC):
                            nc.tensor.matmul(
                                v_ps[t4][:],
                                wkv_sb[:, ci, 128:256],
                                xT_sb[:, ci, _ts(t4, 512)],
                                start=(ci == 0),
                                stop=(ci == CI - 1),
                            )
                    vbb = []
                    for t4 in range(TC):
                        vb = rope_pool.tile([128, 512], BF16, tag=f"vb{t4}")
                        nc.vector.tensor_copy(vb[:], v_ps[t4][:])
                        vbb.append(vb)
                    for t4 in range(TC):
                        rope_chunk(
                            k_ps[t4], t4, bk_sb[:, 0:1],
                            kT_all[:, _ts(t4, 512)], rope_pool,
                        )

                with tc.tile_pool(name="vtp", bufs=2, space="PSUM") as vtp:
                    for t4 in range(TC):
                        for j in range(4):
                            tb = t4 * 4 + j
                            pt = vtp.tile([128, 128], BF16)
                            nc.tensor.transpose(
                                pt[:], vbb[t4][:, _ts(j, 128)], ident_sb[:]
                            )
                            nc.vector.tensor_copy(v_sb[:, tb, :], pt[:])

                with tc.tile_pool(name="qp", bufs=2, space="PSUM") as qp:
                    for h in range(NH):
                        q_ps = [qp.tile([128, 512], F32, name=f"qps{t4}", tag=f"q{t4}") for t4 in range(TC)]
                        for ci in range(CI):
                            for t4 in range(TC):
                                nc.tensor.matmul(
                                    q_ps[t4][:],
                                    wq_sb[:, ci, _ts(h, 128)],
                                    xT_sb[:, ci, _ts(t4, 512)],
                                    start=(ci == 0),
                                    stop=(ci == CI - 1),
                                )
                        for t4 in range(TC):
                            rope_chunk(
                                q_ps[t4], t4, bq_sb[:, h : h + 1],
                                qT_all[:, h, _ts(t4, 512)], rope_pool,
                            )

            # ---- phase 2: attention, pipelined, with o_proj interleave --
            oproj_q = []

            def oproj_group(ti, nj, psum_pool, oe_pool, evict):
                ps = psum_pool.tile([128, 512], F32)
                for h2 in range(NH):
                    nc.tensor.matmul(
                        ps[:],
                        yTn[:, h2, _ts(ti, 128)],
                        wo_sb[:, h2, _ts(nj, 512)],
                        start=(h2 == 0),
                        stop=(h2 == NH - 1),
                    )
                oe = oe_pool.tile([128, 512], F32)
                if evict == "v":
                    nc.vector.tensor_copy(oe[:], ps[:])
                else:
                    nc.scalar.copy(oe[:], ps[:])
                nc.sync.dma_start(
                    out=out_d[_ts(ti, 128), _ts(nj, 512)], in_=oe[:]
                )

            with (
                tc.tile_pool(name="st", bufs=2, space="PSUM") as stp,
                tc.tile_pool(name="yt", bufs=2, space="PSUM") as ytp,
                tc.tile_pool(name="rs", bufs=1, space="PSUM") as rsp,
                tc.tile_pool(name="poi", bufs=1, space="PSUM") as poi,
                tc.tile_pool(name="ptp", bufs=3) as ptp,
                tc.tile_pool(name="rsb", bufs=2) as rsb,
                tc.tile_pool(name="bcb", bufs=2) as bcb,
                tc.tile_pool(name="oei", bufs=3) as oei,
            ):
                def scores_step(h, qc, g):
                    # scores for kb pair (2g, 2g+1) + exp
                    st = stp.tile([128, 1024], F32)
                    for u in range(2):
                        nc.tensor.matmul(
                            st[:, _ts(u, 512)],
                            kT_all[:, _ts(2 * g + u, 128)],
                            qT_all[:, h, _ts(qc, 512)],
                            start=True,
                            stop=True,
                        )
                    pt = ptp.tile([128, 1024], BF16)
                    nc.scalar.activation(pt[:], st[:], Exp, scale=SCALE)
                    return pt

                for qc in range(TC):  # ascending: smallest o_proj tail
                    nkb = 4 * (qc + 1)
                    G = nkb // 2
                    for h in range(NH):
                        yt_ps = ytp.tile([128, 512], F32)
                        rs_ps = rsp.tile([1, 512], F32)
                        pt = scores_step(h, qc, 0)
                        for g in range(G):
                            # pipeline: next step's scores + exp issue first,
                            # so PE has work while ACT computes exp(g)
                            pt_next = (
                                scores_step(h, qc, g + 1) if g + 1 < G else None
                            )
                            # causal mask on diagonal blocks
                            for u in range(2):
                                kb = 2 * g + u
                                j = kb - 4 * qc
                                if j >= 0:
                                    base = u * 512
                                    if j > 0:
                                        nc.vector.memset(
                                            pt[:, base : base + j * 128], 0.0
                                        )
                                    blk = pt[:, base + j * 128 : base + (j + 1) * 128]
                                    nc.vector.tensor_mul(blk, blk, tri_sb[:])
                            for u in range(2):
                                kb = 2 * g + u
                                nc.tensor.matmul(
                                    yt_ps[:],
                                    v_sb[:, kb, :],
                                    pt[:, _ts(u, 512)],
                                    start=(kb == 0),
                                    stop=(kb == nkb - 1),
                                )
                            for u in range(2):
                                kb = 2 * g + u
                                nc.tensor.matmul(
                                    rs_ps[:],
                                    ones_sb[:],
                                    pt[:, _ts(u, 512)],
                                    start=(kb == 0),
                                    stop=(kb == nkb - 1),
                                )
                            if oproj_q:
                                oproj_group(*oproj_q.pop(0), poi, oei, "s")
                            pt = pt_next
                        # head tail: 1/rowsum straight off PSUM (custom DVE),
                        # partition-broadcast on GpSimd, normalize out of PSUM
                        rsv = rsb.tile([1, 512], F32)
                        nc.vector.reciprocal_approx_fast(rsv[:], rs_ps[:])
                        bc = bcb.tile([128, 512], F32)
                        nc.gpsimd.partition_broadcast(bc[:], rsv[:], channels=128)
                        nc.vector.tensor_mul(
                            yTn[:, h, _ts(qc, 512)], yt_ps[:], bc[:]
                        )
                    oproj_q.extend(
                        (ti, nj)
                        for ti in range(4 * qc, 4 * qc + 4)
                        for nj in range(TC)
                    )

            # ---- tail: drain remaining o_proj with a wide PSUM pool -----
            with (
                tc.tile_pool(name="pot", bufs=4, space="PSUM") as pot,
                tc.tile_pool(name="oet", bufs=6) as oet,
            ):
                for i, (ti, nj) in enumerate(oproj_q):
                    oproj_group(ti, nj, pot, oet, "v" if i % 2 == 0 else "s")

    nc.finalize()
    return nc


def _get_program():
    global _PROGRAM
    if _PROGRAM is None:
        _PROGRAM = _build_program()
    return _PROGRAM


def _rope_tables():
    inv_freq = 1.0 / (1000000.0 ** (np.arange(0, D, 2, dtype=np.float64) / D))
    pos = np.arange(T, dtype=np.float64)
    si = np.outer(pos, inv_freq)                      # [T, D/2]
    cos_h, sin_h = np.cos(si), np.sin(si)
    cos = np.stack([cos_h, cos_h], axis=-1).reshape(T, D)
    sin = np.stack([sin_h, sin_h], axis=-1).reshape(T, D)
    cosT = np.ascontiguousarray(cos.T).astype(np.float32)   # [D, T]
    sinT = np.ascontiguousarray(sin.T).astype(np.float32)
    # rotate-half as a partition shift: sh[i<64]=q[i+64], sh[i>=64]=q[i-64];
    # q_rot = q*cos + sh*sin_signed with the -1 for i<64 baked into the table
    sinT[: D // 2] *= -1.0
    return cosT, sinT


def make_in_maps(x, Wq, bq, Wk, bk, Wv, bv, Wo):
    bf = ml_dtypes.bfloat16
    cosT, sinT = _rope_tables()
    tri = np.triu(np.ones((D, D), dtype=np.float32)).astype(bf)  # [k, q]: q >= k
    in_maps = []
    for b in range(2):
        xT = np.ascontiguousarray(x[b].T).astype(bf)
        for g in range(4):
            in_maps.append(
                {
                    "xT": xT,
                    "wq": np.ascontiguousarray(Wq[:, g * 512 : (g + 1) * 512]).astype(bf),
                    "wkv": np.ascontiguousarray(
                        np.concatenate(
                            [
                                Wk[:, g * 128 : (g + 1) * 128],
                                Wv[:, g * 128 : (g + 1) * 128],
                            ],
                            axis=1,
                        )
                    ).astype(bf),
                    "wo": np.ascontiguousarray(Wo[g * 512 : (g + 1) * 512, :]).astype(bf),
                    "bq": np.ascontiguousarray(
                        bq[g * 512 : (g + 1) * 512].reshape(NH, D).T
                    ).astype(np.float32),
                    "bk": np.ascontiguousarray(
                        bk[g * 128 : (g + 1) * 128].reshape(D, 1)
                    ).astype(np.float32),
                    "cosT": cosT.astype(bf),
                    "sinT": sinT.astype(bf),
                    "tri": tri,
                }
            )
    return in_maps


def combine_outputs(res, inputs):
    bv, Wo = np.asarray(inputs["bv"]), np.asarray(inputs["Wo"])
    out = np.zeros((2, T, C), dtype=np.float32)
    for c in range(8):
        g = c % 4
        out[c // 4] += res.results[c]["out"]
        # v-bias contribution: softmax rows sum to 1, so bv adds the constant
        # row (bv tiled over the 4 q heads) @ Wo_group to every output row
        bv_tiled = np.tile(bv[g * 128 : (g + 1) * 128], NH).astype(np.float64)
        cvec = bv_tiled @ Wo[g * 512 : (g + 1) * 512, :].astype(np.float64)
        out[c // 4] += cvec.astype(np.float32)[None, :]
    return out


def kernel(x, Wq, bq, Wk, bk, Wv, bv, Wo):
    nc = _get_program()
    in_maps = make_in_maps(x, Wq, bq, Wk, bk, Wv, bv, Wo)
    res = run_bass_kernel_spmd(nc, in_maps, list(range(8)))
    return combine_outputs(res, {"bv": bv, "Wo": Wo})


# revision 10
# speedup vs baseline: 1.1381x; 1.0088x over previous
"""Trainium2 Bass kernel for causal GQA self-attention (B=2, T=2048, C=2048,
Hq=16, Hkv=4, d=128, RoPE base 1e6).

Sharding: 8 cores = 2 batches x 4 kv-head groups. Each core computes, for its
(batch b, kv group g): the q/k/v projections restricted to that group (4 q
heads + 1 kv head), RoPE, causal attention, and the partial o_proj
(y_group @ Wo[group rows]). The host sums the 4 partial o_proj outputs per
batch (the all-reduce/unshard step of tensor parallelism).

Device schedule (v2):
  - Phase 1 projections run ci-outer / t4-inner with per-t4 PSUM banks so a
    weight chunk is stationary across 4 back-to-back matmuls, and k+v are
    interleaved per-ci so PE consumption (~1.7us/ci) matches DMA arrival
    (~1.6us/ci) from the first microsecond.
  - RoPE runs fully in bf16 (cos/sin tables shipped bf16) for 2x DVE rate.
  - Attention is software-pipelined: scores for step g+1 issue before the
    AV/rowsum matmuls of step g, so the ACT exp latency hides entirely under
    PE work. Scores are computed transposed (S^T = k @ qT) so exp needs no
    cross-partition work.
  - softmax 1/rowsum: rowsums accumulate via a ones-vector matmul in PSUM;
    the reciprocal runs as one custom-DVE op straight out of PSUM, is
    broadcast across partitions by the (otherwise idle) GpSimd engine, and
    multiplies yT straight out of its PSUM bank -- no DRAM round trip, no
    ACT log/exp chain.
  - o_proj work is queued and interleaved one (ti,nj) group per attention
    step (covers per-head normalization chains); qc runs ascending so only
    the last qc's o_proj groups remain for the tail, processed with a 4-bank
    PSUM pool.
  - the v bias is folded out entirely: since softmax rows sum to 1, bv
    contributes the constant row bv_tiled @ Wo_g, added on the host.
"""

import numpy as np
import ml_dtypes

import concourse.bass as bass
import concourse.mybir as mybir
from concourse import bacc
from concourse.tile import TileContext
from concourse.bass_utils import run_bass_kernel_spmd
from concourse.masks import make_identity

BF16 = mybir.dt.bfloat16
F32 = mybir.dt.float32

T = 2048
C = 2048
D = 128
NH = 4           # q heads per core
CI = C // 128    # contraction chunks
TC = T // 512    # t chunks of 512
TB = T // 128    # t blocks of 128
SCALE = 1.0 / np.sqrt(D)

_PROGRAM = None


def _ts(i, s):
    return bass.ts(i, s)


def _patch_act_tables():
    """Force every ACT function this kernel uses to resolve to the
    natural_log_exp_and_others table set, so the whole kernel needs exactly
    one ACT_TABLE_LOAD. Returns an undo callable."""
    import concourse.bacc as bacc_mod

    orig = bacc_mod.get_activation_tables
    A = mybir.ActivationFunctionType
    mine = {A.Exp, A.Ln, A.Identity, A.Copy}

    def patched(arch):
        tables = dict(orig(arch))
        for name in tables:
            if name != "natural_log_exp_and_others":
                tables[name] = set(tables[name]) - mine
        return tables

    bacc_mod.get_activation_tables = patched

    def undo():
        bacc_mod.get_activation_tables = orig

    return undo


def _build_program():
    undo = _patch_act_tables()
    try:
        return _build_program_inner()
    finally:
        undo()


def _build_program_inner():
    nc = bacc.Bacc("TRN2", target_bir_lowering=False, debug=False, num_devices=8)

    xT_d = nc.dram_tensor("xT", [C, T], BF16, kind="ExternalInput").ap()
    wq_d = nc.dram_tensor("wq", [C, NH * D], BF16, kind="ExternalInput").ap()
    wkv_d = nc.dram_tensor("wkv", [C, 2 * D], BF16, kind="ExternalInput").ap()
    wo_d = nc.dram_tensor("wo", [NH * D, C], BF16, kind="ExternalInput").ap()
    bq_d = nc.dram_tensor("bq", [D, NH], F32, kind="ExternalInput").ap()
    bk_d = nc.dram_tensor("bk", [D, 1], F32, kind="ExternalInput").ap()
    cos_d = nc.dram_tensor("cosT", [D, T], BF16, kind="ExternalInput").ap()
    sin_d = nc.dram_tensor("sinT", [D, T], BF16, kind="ExternalInput").ap()
    tri_d = nc.dram_tensor("tri", [D, D], BF16, kind="ExternalInput").ap()
    out_d = nc.dram_tensor("out", [T, C], F32, kind="ExternalOutput").ap()

    Ident = mybir.ActivationFunctionType.Identity
    Exp = mybir.ActivationFunctionType.Exp

    with TileContext(nc) as tc:
        with (
            tc.tile_pool(name="consts", bufs=1) as consts,
            tc.tile_pool(name="acts", bufs=1) as acts,
        ):
            # ---- resident constants, DMAs issued in consumption order ----
            xT_sb = consts.tile([128, CI, T], BF16)
            wq_sb = consts.tile([128, CI, NH * D], BF16)
            wkv_sb = consts.tile([128, CI, 2 * D], BF16)
            wo_sb = consts.tile([128, NH, C], BF16)
            bq_sb = consts.tile([128, NH], F32)
            bk_sb = consts.tile([128, 1], F32)
            cos_sb = consts.tile([128, T], BF16)
            sin_sb = consts.tile([128, T], BF16)
            tri_sb = consts.tile([128, 128], BF16)
            ones_sb = consts.tile([128, 1], BF16)
            ident_sb = consts.tile([128, 128], BF16)

            nc.sync.dma_start(out=bk_sb[:], in_=bk_d[:])
            nc.sync.dma_start(out=bq_sb[:], in_=bq_d[:])
            for ci in range(CI):
                nc.sync.dma_start(out=wkv_sb[:, ci, :], in_=wkv_d[_ts(ci, 128), :])
                nc.sync.dma_start(out=xT_sb[:, ci, :], in_=xT_d[_ts(ci, 128), :])
            for ci in range(CI):
                nc.sync.dma_start(out=wq_sb[:, ci, :], in_=wq_d[_ts(ci, 128), :])
            nc.sync.dma_start(out=cos_sb[:], in_=cos_d[:])
            nc.sync.dma_start(out=sin_sb[:], in_=sin_d[:])
            nc.sync.dma_start(out=tri_sb[:], in_=tri_d[:])
            for h in range(NH):
                nc.sync.dma_start(out=wo_sb[:, h, :], in_=wo_d[_ts(h, 128), :])
            nc.vector.memset(ones_sb[:], 1.0)
            make_identity(nc, ident_sb[:])

            # ---- persistent activations ---------------------------------
            qT_all = acts.tile([128, NH, T], BF16)   # rotated q^T per head
            kT_all = acts.tile([128, T], BF16)       # rotated k^T
            v_sb = acts.tile([128, TB, D], BF16)     # v in natural [t, d] blocks
            yTn = acts.tile([128, NH, T], BF16)      # normalized y^T per head

            def rope_chunk(ps, t4, bias_ap, dest, pool):
                # psum -> bf16 with bias; rotate-half done as two ACT
                # half-evictions straight from PSUM (partition swap, sign
                # baked into the sin table), then cos/sin muls on DVE
                qb = pool.tile([128, 512], BF16, tag="qb")
                nc.scalar.activation(qb[:], ps[:], Ident, bias=bias_ap)
                sh = pool.tile([128, 512], BF16, tag="sh")
                nc.scalar.copy(sh[0:64, :], qb[64:128, :])
                nc.scalar.copy(sh[64:128, :], qb[0:64, :])
                t1 = pool.tile([128, 512], BF16, tag="t1")
                nc.vector.tensor_mul(t1[:], qb[:], cos_sb[:, _ts(t4, 512)])
                nc.vector.tensor_mul(sh[:], sh[:], sin_sb[:, _ts(t4, 512)])
                nc.vector.tensor_add(dest, t1[:], sh[:])

            # ---- phase 1: projections ----------------------------------
            # k+v interleaved per-ci (8 PSUM banks; PE ~1.7us/ci matches the
            # 2-dispatch DMA stream), then v transposes, then q0..q3.
            with tc.tile_pool(name="rope", bufs=4) as rope_pool:
                with (
                    tc.tile_pool(name="kp", bufs=1, space="PSUM") as kp,
                    tc.tile_pool(name="vp", bufs=1, space="PSUM") as vp,
                ):
                    k_ps = [kp.tile([128, 512], F32, name=f"kps{t4}", tag=f"k{t4}") for t4 in range(TC)]
                    v_ps = [vp.tile([128, 512], F32, name=f"vps{t4}", tag=f"v{t4}") for t4 in range(TC)]
                    for ci in range(CI):
                        for t4 in range(TC):
                            nc.tensor.matmul(
                                k_ps[t4][:],
                                wkv_sb[:, ci, 0:128],
                                xT_sb[:, ci, _ts(t4, 512)],
                                start=(ci == 0),
                                stop=(ci == CI - 1),
                            )
                        for t4 in range(TC):
                            nc.tensor.matmul(
                                v_ps[t4][:],
                                wkv_sb[:, ci, 128:256],
                                xT_sb[:, ci, _ts(t4, 512)],
                                start=(ci == 0),
                                stop=(ci == CI - 1),
                            )
                    vbb = []
                    for t4 in range(TC):
                        vb = rope_pool.tile([128, 512], BF16, tag=f"vb{t4}")
                        nc.vector.tensor_copy(vb[:], v_ps[t4][:])
                        vbb.append(vb)
                    for t4 in range(TC):
                        rope_chunk(
                            k_ps[t4], t4, bk_sb[:, 0:1],
                            kT_all[:, _ts(t4, 512)], rope_pool,
                        )

                with tc.tile_pool(name="vtp", bufs=2, space="PSUM") as vtp:
                    for t4 in range(TC):
                        for j in range(4):
                            tb = t4 * 4 + j
                            pt = vtp.tile([128, 128], BF16)
                            nc.tensor.transpose(
                                pt[:], vbb[t4][:, _ts(j, 128)], ident_sb[:]
                            )
                            nc.vector.tensor_copy(v_sb[:, tb, :], pt[:])

                with tc.tile_pool(name="qp", bufs=2, space="PSUM") as qp:
                    for h in range(NH):
                        q_ps = [qp.tile([128, 512], F32, name=f"qps{t4}", tag=f"q{t4}") for t4 in range(TC)]
                        for ci in range(CI):
                            for t4 in range(TC):
                                nc.tensor.matmul(
                                    q_ps[t4][:],
                                    wq_sb[:, ci, _ts(h, 128)],
                                    xT_sb[:, ci, _ts(t4, 512)],
                                    start=(ci == 0),
                                    stop=(ci == CI - 1),
                                )
                        for t4 in range(TC):
                            rope_chunk(
                                q_ps[t4], t4, bq_sb[:, h : h + 1],
                                qT_all[:, h, _ts(t4, 512)], rope_pool,
                            )

            # ---- phase 2: attention, pipelined, with o_proj interleave --
            oproj_q = []

            def oproj_group(ti, nj, psum_pool, oe_pool, evict):
                ps = psum_pool.tile([128, 512], F32)
                for h2 in range(NH):
                    nc.tensor.matmul(
                        ps[:],
                        yTn[:, h2, _ts(ti, 128)],
                        wo_sb[:, h2, _ts(nj, 512)],
                        start=(h2 == 0),
                        stop=(h2 == NH - 1),
                    )
                oe = oe_pool.tile([128, 512], F32)
                if evict == "v":
                    nc.vector.tensor_copy(oe[:], ps[:])
                else:
                    nc.scalar.copy(oe[:], ps[:])
                nc.sync.dma_start(
                    out=out_d[_ts(ti, 128), _ts(nj, 512)], in_=oe[:]
                )

            with (
                tc.tile_pool(name="st", bufs=2, space="PSUM") as stp,
                tc.tile_pool(name="yt", bufs=2, space="PSUM") as ytp,
                tc.tile_pool(name="rs", bufs=1, space="PSUM") as rsp,
                tc.tile_pool(name="poi", bufs=1, space="PSUM") as poi,
                tc.tile_pool(name="ptp", bufs=3) as ptp,
                tc.tile_pool(name="rsb", bufs=2) as rsb,
                tc.tile_pool(name="bcb", bufs=2) as bcb,
                tc.tile_pool(name="oei", bufs=3) as oei,
            ):
                def scores_step(h, qc, g):
                    # scores for kb pair (2g, 2g+1) + exp
                    st = stp.tile([128, 1024], F32)
                    for u in range(2):
                        nc.tensor.matmul(
                            st[:, _ts(u, 512)],
                            kT_all[:, _ts(2 * g + u, 128)],
                            qT_all[:, h, _ts(qc, 512)],
                            start=True,
                            stop=True,
                        )
                    pt = ptp.tile([128, 1024], BF16)
                    nc.scalar.activation(pt[:], st[:], Exp, scale=SCALE)
                    return pt

                for qc in range(TC):  # ascending: smallest o_proj tail
                    nkb = 4 * (qc + 1)
                    G = nkb // 2
                    for h in range(NH):
                        yt_ps = ytp.tile([128, 512], F32)
                        rs_ps = rsp.tile([1, 512], F32)
                        pt = scores_step(h, qc, 0)
                        for g in range(G):
                            # pipeline: next step's scores + exp issue first,
                            # so PE has work while ACT computes exp(g)
                            pt_next = (
                                scores_step(h, qc, g + 1) if g + 1 < G else None
                            )
                            # causal mask on diagonal blocks
                            for u in range(2):
                                kb = 2 * g + u
                                j = kb - 4 * qc
                                if j >= 0:
                                    base = u * 512
                                    if j > 0:
                                        nc.vector.memset(
                                            pt[:, base : base + j * 128], 0.0
                                        )
                                    blk = pt[:, base + j * 128 : base + (j + 1) * 128]
                                    nc.vector.tensor_mul(blk, blk, tri_sb[:])
                            for u in range(2):
                                kb = 2 * g + u
                                nc.tensor.matmul(
                                    yt_ps[:],
                                    v_sb[:, kb, :],
                                    pt[:, _ts(u, 512)],
                                    start=(kb == 0),
                                    stop=(kb == nkb - 1),
                                )
                            for u in range(2):
                                kb = 2 * g + u
                                nc.tensor.matmul(
                                    rs_ps[:],
                                    ones_sb[:],
                                    pt[:, _ts(u, 512)],
                                    start=(kb == 0),
                                    stop=(kb == nkb - 1),
                                )
                            if oproj_q:
                                oproj_group(*oproj_q.pop(0), poi, oei, "s")
                            pt = pt_next
                        # head tail: 1/rowsum straight off PSUM (custom DVE),
                        # partition-broadcast on GpSimd, normalize out of PSUM
                        rsv = rsb.tile([1, 512], F32)
                        nc.vector.reciprocal_approx_fast(rsv[:], rs_ps[:])
                        bc = bcb.tile([128, 512], F32)
                        nc.gpsimd.partition_broadcast(bc[:], rsv[:], channels=128)
                        nc.vector.tensor_mul(
                            yTn[:, h, _ts(qc, 512)], yt_ps[:], bc[:]
                        )
                    oproj_q.extend(
                        (ti, nj)
                        for ti in range(4 * qc, 4 * qc + 4)
                        for nj in range(TC)
                    )

            # ---- tail: drain remaining o_proj with a wide PSUM pool -----
            with (
                tc.tile_pool(name="pot", bufs=4, space="PSUM") as pot,
                tc.tile_pool(name="oet", bufs=6) as oet,
            ):
                for i, (ti, nj) in enumerate(oproj_q):
                    oproj_group(ti, nj, pot, oet, "v" if i % 2 == 0 else "s")

    nc.finalize()
    return nc


def _get_program():
    global _PROGRAM
    if _PROGRAM is None:
        _PROGRAM = _build_program()
    return _PROGRAM


def _rope_tables():
    inv_freq = 1.0 / (1000000.0 ** (np.arange(0, D, 2, dtype=np.float64) / D))
    pos = np.arange(T, dtype=np.float64)
    si = np.outer(pos, inv_freq)                      # [T, D/2]
    cos_h, sin_h = np.cos(si), np.sin(si)
    cos = np.stack([cos_h, cos_h], axis=-1).reshape(T, D)
    sin = np.stack([sin_h, sin_h], axis=-1).reshape(T, D)
    cosT = np.ascontiguousarray(cos.T).astype(np.float32)   # [D, T]
    sinT = np.ascontiguousarray(sin.T).astype(np.float32)
    # rotate-half as a partition shift: sh[i<64]=q[i+64], sh[i>=64]=q[i-64];
    # q_rot = q*cos + sh*sin_signed with the -1 for i<64 baked into the table
    sinT[: D // 2] *= -1.0
    return cosT, sinT


def make_in_maps(x, Wq, bq, Wk, bk, Wv, bv, Wo):
    bf = ml_dtypes.bfloat16
    cosT, sinT = _rope_tables()
    tri = np.triu(np.ones((D, D), dtype=np.float32)).astype(bf)  # [k, q]: q >= k
    in_maps = []
    for b in range(2):
        xT = np.ascontiguousarray(x[b].T).astype(bf)
        for g in range(4):
            in_maps.append(
                {
                    "xT": xT,
                    "wq": np.ascontiguousarray(Wq[:, g * 512 : (g + 1) * 512]).astype(bf),
                    "wkv": np.ascontiguousarray(
                        np.concatenate(
                            [
                                Wk[:, g * 128 : (g + 1) * 128],
                                Wv[:, g * 128 : (g + 1) * 128],
                            ],
                            axis=1,
                        )
                    ).astype(bf),
                    "wo": np.ascontiguousarray(Wo[g * 512 : (g + 1) * 512, :]).astype(bf),
                    "bq": np.ascontiguousarray(
                        bq[g * 512 : (g + 1) * 512].reshape(NH, D).T
                    ).astype(np.float32),
                    "bk": np.ascontiguousarray(
                        bk[g * 128 : (g + 1) * 128].reshape(D, 1)
                    ).astype(np.float32),
                    "cosT": cosT.astype(bf),
                    "sinT": sinT.astype(bf),
                    "tri": tri,
                }
            )
    return in_maps


def combine_outputs(res, inputs):
    bv, Wo = np.asarray(inputs["bv"]), np.asarray(inputs["Wo"])
    out = np.zeros((2, T, C), dtype=np.float32)
    for c in range(8):
        g = c % 4
        out[c // 4] += res.results[c]["out"]
        # v-bias contribution: softmax rows sum to 1, so bv adds the constant
        # row (bv tiled over the 4 q heads) @ Wo_group to every output row
        bv_tiled = np.tile(bv[g * 128 : (g + 1) * 128], NH).astype(np.float64)
        cvec = bv_tiled @ Wo[g * 512 : (g + 1) * 512, :].astype(np.float64)
        out[c // 4] += cvec.astype(np.float32)[None, :]
    return out


def kernel(x, Wq, bq, Wk, bk, Wv, bv, Wo):
    nc = _get_program()
    in_maps = make_in_maps(x, Wq, bq, Wk, bk, Wv, bv, Wo)
    res = run_bass_kernel_spmd(nc, in_maps, list(range(8)))
    return combine_outputs(res, {"bv": bv, "Wo": Wo})


# revision 12
# speedup vs baseline: 1.1440x; 1.0053x over previous
"""Trainium2 Bass kernel for causal GQA self-attention (B=2, T=2048, C=2048,
Hq=16, Hkv=4, d=128, RoPE base 1e6).

Sharding: 8 cores = 2 batches x 4 kv-head groups. Each core computes, for its
(batch b, kv group g): the q/k/v projections restricted to that group (4 q
heads + 1 kv head), RoPE, causal attention, and the partial o_proj
(y_group @ Wo[group rows]). The host sums the 4 partial o_proj outputs per
batch (the all-reduce/unshard step of tensor parallelism).

Device schedule (v2):
  - Phase 1 projections run ci-outer / t4-inner with per-t4 PSUM banks so a
    weight chunk is stationary across 4 back-to-back matmuls, and k+v are
    interleaved per-ci so PE consumption (~1.7us/ci) matches DMA arrival
    (~1.6us/ci) from the first microsecond.
  - RoPE runs fully in bf16 (cos/sin tables shipped bf16) for 2x DVE rate.
  - Attention is software-pipelined: scores for step g+1 issue before the
    AV/rowsum matmuls of step g, so the ACT exp latency hides entirely under
    PE work. Scores are computed transposed (S^T = k @ qT) so exp needs no
    cross-partition work.
  - softmax 1/rowsum: rowsums accumulate via a ones-vector matmul in PSUM;
    the reciprocal runs as one custom-DVE op straight out of PSUM, is
    broadcast across partitions by the (otherwise idle) GpSimd engine, and
    multiplies yT straight out of its PSUM bank -- no DRAM round trip, no
    ACT log/exp chain.
  - o_proj work is queued and interleaved one (ti,nj) group per attention
    step (covers per-head normalization chains); qc runs ascending so only
    the last qc's o_proj groups remain for the tail, processed with a 4-bank
    PSUM pool.
  - the v bias is folded out entirely: since softmax rows sum to 1, bv
    contributes the constant row bv_tiled @ Wo_g, added on the host.
"""

import numpy as np
import ml_dtypes

import concourse.bass as bass
import concourse.mybir as mybir
from concourse import bacc
from concourse.tile import TileContext
from concourse.bass_utils import run_bass_kernel_spmd
from concourse.masks import make_identity

BF16 = mybir.dt.bfloat16
F32 = mybir.dt.float32

T = 2048
C = 2048
D = 128
NH = 4           # q heads per core
CI = C // 128    # contraction chunks
TC = T // 512    # t chunks of 512
TB = T // 128    # t blocks of 128
SCALE = 1.0 / np.sqrt(D)

_PROGRAM = None


def _ts(i, s):
    return bass.ts(i, s)


def _patch_act_tables():
    """Force every ACT function this kernel uses to resolve to the
    natural_log_exp_and_others table set, so the whole kernel needs exactly
    one ACT_TABLE_LOAD. Returns an undo callable."""
    import concourse.bacc as bacc_mod

    orig = bacc_mod.get_activation_tables
    A = mybir.ActivationFunctionType
    mine = {A.Exp, A.Ln, A.Identity, A.Copy}

    def patched(arch):
        tables = dict(orig(arch))
        for name in tables:
            if name != "natural_log_exp_and_others":
                tables[name] = set(tables[name]) - mine
        return tables

    bacc_mod.get_activation_tables = patched

    def undo():
        bacc_mod.get_activation_tables = orig

    return undo


def _build_program():
    undo = _patch_act_tables()
    try:
        return _build_program_inner()
    finally:
        undo()


def _build_program_inner():
    nc = bacc.Bacc("TRN2", target_bir_lowering=False, debug=False, num_devices=8)

    xT_d = nc.dram_tensor("xT", [C, T], BF16, kind="ExternalInput").ap()
    wq_d = nc.dram_tensor("wq", [C, NH * D], BF16, kind="ExternalInput").ap()
    wkv_d = nc.dram_tensor("wkv", [C, 2 * D], BF16, kind="ExternalInput").ap()
    wo_d = nc.dram_tensor("wo", [NH * D, C], BF16, kind="ExternalInput").ap()
    bq_d = nc.dram_tensor("bq", [D, NH], F32, kind="ExternalInput").ap()
    bk_d = nc.dram_tensor("bk", [D, 1], F32, kind="ExternalInput").ap()
    cos_d = nc.dram_tensor("cosT", [D, T], BF16, kind="ExternalInput").ap()
    sin_d = nc.dram_tensor("sinT", [D, T], BF16, kind="ExternalInput").ap()
    tri_d = nc.dram_tensor("tri", [D, D], BF16, kind="ExternalInput").ap()
    out_d = nc.dram_tensor("out", [T, C], F32, kind="ExternalOutput").ap()

    Ident = mybir.ActivationFunctionType.Identity
    Exp = mybir.ActivationFunctionType.Exp

    with TileContext(nc) as tc:
        with (
            tc.tile_pool(name="consts", bufs=1) as consts,
            tc.tile_pool(name="acts", bufs=1) as acts,
        ):
            # ---- resident constants, DMAs issued in consumption order ----
            xT_sb = consts.tile([128, CI, T], BF16)
            wq_sb = consts.tile([128, CI, NH * D], BF16)
            wkv_sb = consts.tile([128, CI, 2 * D], BF16)
            wo_sb = consts.tile([128, NH, C], BF16)
            bq_sb = consts.tile([128, NH], F32)
            bk_sb = consts.tile([128, 1], F32)
            cos_sb = consts.tile([128, T], BF16)
            sin_sb = consts.tile([128, T], BF16)
            tri_sb = consts.tile([128, 128], BF16)
            ones_sb = consts.tile([128, 1], BF16)
            ident_sb = consts.tile([128, 128], BF16)

            nc.sync.dma_start(out=bk_sb[:], in_=bk_d[:])
            nc.sync.dma_start(out=bq_sb[:], in_=bq_d[:])
            for ci in range(CI):
                nc.sync.dma_start(out=wkv_sb[:, ci, :], in_=wkv_d[_ts(ci, 128), :])
                nc.sync.dma_start(out=xT_sb[:, ci, :], in_=xT_d[_ts(ci, 128), :])
            for ci in range(CI):
                nc.sync.dma_start(out=wq_sb[:, ci, :], in_=wq_d[_ts(ci, 128), :])
            nc.sync.dma_start(out=cos_sb[:], in_=cos_d[:])
            nc.sync.dma_start(out=sin_sb[:], in_=sin_d[:])
            nc.sync.dma_start(out=tri_sb[:], in_=tri_d[:])
            for h in range(NH):
                nc.sync.dma_start(out=wo_sb[:, h, :], in_=wo_d[_ts(h, 128), :])
            nc.vector.memset(ones_sb[:], 1.0)
            make_identity(nc, ident_sb[:])

            # ---- persistent activations ---------------------------------
            qT_all = acts.tile([128, NH, T], BF16)   # rotated q^T per head
            kT_all = acts.tile([128, T], BF16)       # rotated k^T
            v_sb = acts.tile([128, TB, D], BF16)     # v in natural [t, d] blocks
            yTn = acts.tile([128, NH, T], BF16)      # normalized y^T per head

            def rope_evict(ps_list, bias_ap, pool, tagp):
                # evict all chunks first: the 4th IDENT is what releases the
                # PSUM pool for the next phase, so do not interleave swaps
                qbs = []
                for t4, ps in enumerate(ps_list):
                    qb = qbp.tile([128, 512], BF16, name=f"{tagp}qb{t4}", tag=f"{tagp}qb{t4}")
                    nc.scalar.activation(qb[:], ps[:], Ident, bias=bias_ap)
                    qbs.append(qb)
                return qbs

            def rope_rotate(qbs, dest_of, pool):
                # rotate-half as two ACT half-copies (partition swap, sign
                # baked into the sin table), then cos/sin muls on DVE
                for t4, qb in enumerate(qbs):
                    sh = pool.tile([128, 512], BF16, tag="sh")
                    nc.scalar.copy(sh[0:64, :], qb[64:128, :])
                    nc.scalar.copy(sh[64:128, :], qb[0:64, :])
                    t1 = pool.tile([128, 512], BF16, tag="t1")
                    nc.vector.tensor_mul(t1[:], qb[:], cos_sb[:, _ts(t4, 512)])
                    nc.vector.tensor_mul(sh[:], sh[:], sin_sb[:, _ts(t4, 512)])
                    nc.vector.tensor_add(dest_of(t4), t1[:], sh[:])

            # ---- phase 1: projections ----------------------------------
            # k+v interleaved per-ci (8 PSUM banks; PE ~1.7us/ci matches the
            # 2-dispatch DMA stream), then v transposes, then q0..q3.
            with (
                tc.tile_pool(name="rope", bufs=4) as rope_pool,
                tc.tile_pool(name="qbp", bufs=1) as qbp,
            ):
                with (
                    tc.tile_pool(name="kp", bufs=1, space="PSUM") as kp,
                    tc.tile_pool(name="vp", bufs=1, space="PSUM") as vp,
                ):
                    k_ps = [kp.tile([128, 512], F32, name=f"kps{t4}", tag=f"k{t4}") for t4 in range(TC)]
                    v_ps = [vp.tile([128, 512], F32, name=f"vps{t4}", tag=f"v{t4}") for t4 in range(TC)]
                    for ci in range(CI):
                        for t4 in range(TC):
                            nc.tensor.matmul(
                                k_ps[t4][:],
                                wkv_sb[:, ci, 0:128],
                                xT_sb[:, ci, _ts(t4, 512)],
                                start=(ci == 0),
                                stop=(ci == CI - 1),
                            )
                        for t4 in range(TC):
                            nc.tensor.matmul(
                                v_ps[t4][:],
                                wkv_sb[:, ci, 128:256],
                                xT_sb[:, ci, _ts(t4, 512)],
                                start=(ci == 0),
                                stop=(ci == CI - 1),
                            )
                    vbb = []
                    for t4 in range(TC):
                        vb = qbp.tile([128, 512], BF16, name=f"vb{t4}", tag=f"vb{t4}")
                        nc.vector.tensor_copy(vb[:], v_ps[t4][:])
                        vbb.append(vb)
                    k_qbs = rope_evict(k_ps, bk_sb[:, 0:1], rope_pool, "k")
                    rope_rotate(
                        k_qbs, lambda t4: kT_all[:, _ts(t4, 512)], rope_pool
                    )

                with tc.tile_pool(name="vtp", bufs=2, space="PSUM") as vtp:
                    for t4 in range(TC):
                        for j in range(4):
                            tb = t4 * 4 + j
                            pt = vtp.tile([128, 128], BF16)
                            nc.tensor.transpose(
                                pt[:], vbb[t4][:, _ts(j, 128)], ident_sb[:]
                            )
                            nc.vector.tensor_copy(v_sb[:, tb, :], pt[:])

                with tc.tile_pool(name="qp", bufs=2, space="PSUM") as qp:
                    for h in range(NH):
                        q_ps = [qp.tile([128, 512], F32, name=f"qps{t4}", tag=f"q{t4}") for t4 in range(TC)]
                        for ci in range(CI):
                            for t4 in range(TC):
                                nc.tensor.matmul(
                                    q_ps[t4][:],
                                    wq_sb[:, ci, _ts(h, 128)],
                                    xT_sb[:, ci, _ts(t4, 512)],
                                    start=(ci == 0),
                                    stop=(ci == CI - 1),
                                )
                        q_qbs = rope_evict(q_ps, bq_sb[:, h : h + 1], rope_pool, f"q{h}")
                        rope_rotate(
                            q_qbs,
                            lambda t4, h=h: qT_all[:, h, _ts(t4, 512)],
                            rope_pool,
                        )

            # ---- phase 2: attention, pipelined, with o_proj interleave --
            oproj_q = []

            def oproj_group(ti, nj, psum_pool, oe_pool, evict):
                ps = psum_pool.tile([128, 512], F32)
                for h2 in range(NH):
                    nc.tensor.matmul(
                        ps[:],
                        yTn[:, h2, _ts(ti, 128)],
                        wo_sb[:, h2, _ts(nj, 512)],
                        start=(h2 == 0),
                        stop=(h2 == NH - 1),
                    )
                oe = oe_pool.tile([128, 512], F32)
                if evict == "v":
                    nc.vector.tensor_copy(oe[:], ps[:])
                else:
                    nc.scalar.copy(oe[:], ps[:])
                nc.sync.dma_start(
                    out=out_d[_ts(ti, 128), _ts(nj, 512)], in_=oe[:]
                )

            with (
                tc.tile_pool(name="st", bufs=2, space="PSUM") as stp,
                tc.tile_pool(name="yt", bufs=2, space="PSUM") as ytp,
                tc.tile_pool(name="rs", bufs=1, space="PSUM") as rsp,
                tc.tile_pool(name="poi", bufs=1, space="PSUM") as poi,
                tc.tile_pool(name="ptp", bufs=3) as ptp,
                tc.tile_pool(name="rsb", bufs=2) as rsb,
                tc.tile_pool(name="bcb", bufs=2) as bcb,
                tc.tile_pool(name="oei", bufs=3) as oei,
            ):
                def scores_step(h, qc, g):
                    # scores for kb pair (2g, 2g+1) + exp
                    st = stp.tile([128, 1024], F32)
                    for u in range(2):
                        nc.tensor.matmul(
                            st[:, _ts(u, 512)],
                            kT_all[:, _ts(2 * g + u, 128)],
                            qT_all[:, h, _ts(qc, 512)],
                            start=True,
                            stop=True,
                        )
                    pt = ptp.tile([128, 1024], BF16)
                    nc.scalar.activation(pt[:], st[:], Exp, scale=SCALE)
                    return pt

                for qc in range(TC):  # ascending: smallest o_proj tail
                    nkb = 4 * (qc + 1)
                    G = nkb // 2
                    for h in range(NH):
                        yt_ps = ytp.tile([128, 512], F32)
                        rs_ps = rsp.tile([1, 512], F32)
                        pt = scores_step(h, qc, 0)
                        for g in range(G):
                            # pipeline: next step's scores + exp issue first,
                            # so PE has work while ACT computes exp(g)
                            pt_next = (
                                scores_step(h, qc, g + 1) if g + 1 < G else None
                            )
                            # causal mask on diagonal blocks
                            for u in range(2):
                                kb = 2 * g + u
                                j = kb - 4 * qc
                                if j >= 0:
                                    base = u * 512
                                    if j > 0:
                                        nc.vector.memset(
                                            pt[:, base : base + j * 128], 0.0
                                        )
                                    blk = pt[:, base + j * 128 : base + (j + 1) * 128]
                                    nc.vector.tensor_mul(blk, blk, tri_sb[:])
                            for u in range(2):
                                kb = 2 * g + u
                                nc.tensor.matmul(
                                    yt_ps[:],
                                    v_sb[:, kb, :],
                                    pt[:, _ts(u, 512)],
                                    start=(kb == 0),
                                    stop=(kb == nkb - 1),
                                )
                            for u in range(2):
                                kb = 2 * g + u
                                nc.tensor.matmul(
                                    rs_ps[:],
                                    ones_sb[:],
                                    pt[:, _ts(u, 512)],
                                    start=(kb == 0),
                                    stop=(kb == nkb - 1),
                                )
                            if oproj_q and h > 0:
                                oproj_group(*oproj_q.pop(0), poi, oei, "s")
                            pt = pt_next
                        # head tail: 1/rowsum straight off PSUM (custom DVE),
                        # partition-broadcast on GpSimd, normalize out of PSUM
                        rsv = rsb.tile([1, 512], F32)
                        nc.vector.reciprocal_approx_fast(rsv[:], rs_ps[:])
                        bc = bcb.tile([128, 512], F32)
                        nc.gpsimd.partition_broadcast(bc[:], rsv[:], channels=128)
                        nc.vector.tensor_mul(
                            yTn[:, h, _ts(qc, 512)], yt_ps[:], bc[:]
                        )
                    oproj_q.extend(
                        (ti, nj)
                        for ti in range(4 * qc, 4 * qc + 4)
                        for nj in range(TC)
                    )

            # ---- tail: drain remaining o_proj with a wide PSUM pool -----
            with (
                tc.tile_pool(name="pot", bufs=4, space="PSUM") as pot,
                tc.tile_pool(name="oet", bufs=6) as oet,
            ):
                for i, (ti, nj) in enumerate(oproj_q):
                    oproj_group(ti, nj, pot, oet, "v" if i % 2 == 0 else "s")

    nc.finalize()
    return nc


def _get_program():
    global _PROGRAM
    if _PROGRAM is None:
        _PROGRAM = _build_program()
    return _PROGRAM


def _rope_tables():
    inv_freq = 1.0 / (1000000.0 ** (np.arange(0, D, 2, dtype=np.float64) / D))
    pos = np.arange(T, dtype=np.float64)
    si = np.outer(pos, inv_freq)                      # [T, D/2]
    cos_h, sin_h = np.cos(si), np.sin(si)
    cos = np.stack([cos_h, cos_h], axis=-1).reshape(T, D)
    sin = np.stack([sin_h, sin_h], axis=-1).reshape(T, D)
    cosT = np.ascontiguousarray(cos.T).astype(np.float32)   # [D, T]
    sinT = np.ascontiguousarray(sin.T).astype(np.float32)
    # rotate-half as a partition shift: sh[i<64]=q[i+64], sh[i>=64]=q[i-64];
    # q_rot = q*cos + sh*sin_signed with the -1 for i<64 baked into the table
    sinT[: D // 2] *= -1.0
    return cosT, sinT


def make_in_maps(x, Wq, bq, Wk, bk, Wv, bv, Wo):
    bf = ml_dtypes.bfloat16
    cosT, sinT = _rope_tables()
    tri = np.triu(np.ones((D, D), dtype=np.float32)).astype(bf)  # [k, q]: q >= k
    in_maps = []
    for b in range(2):
        xT = np.ascontiguousarray(x[b].T).astype(bf)
        for g in range(4):
            in_maps.append(
                {
                    "xT": xT,
                    "wq": np.ascontiguousarray(Wq[:, g * 512 : (g + 1) * 512]).astype(bf),
                    "wkv": np.ascontiguousarray(
                        np.concatenate(
                            [
                                Wk[:, g * 128 : (g + 1) * 128],
                                Wv[:, g * 128 : (g + 1) * 128],
                            ],
                            axis=1,
                        )
                    ).astype(bf),
                    "wo": np.ascontiguousarray(Wo[g * 512 : (g + 1) * 512, :]).astype(bf),
                    "bq": np.ascontiguousarray(
                        bq[g * 512 : (g + 1) * 512].reshape(NH, D).T
                    ).astype(np.float32),
                    "bk": np.ascontiguousarray(
                        bk[g * 128 : (g + 1) * 128].reshape(D, 1)
                    ).astype(np.float32),
                    "cosT": cosT.astype(bf),
                    "sinT": sinT.astype(bf),
                    "tri": tri,
                }
            )
    return in_maps


def combine_outputs(res, inputs):
    bv, Wo = np.asarray(inputs["bv"]), np.asarray(inputs["Wo"])
    out = np.zeros((2, T, C), dtype=np.float32)
    for c in range(8):
        g = c % 4
        out[c // 4] += res.results[c]["out"]
        # v-bias contribution: softmax rows sum to 1, so bv adds the constant
        # row (bv tiled over the 4 q heads) @ Wo_group to every output row
        bv_tiled = np.tile(bv[g * 128 : (g + 1) * 128], NH).astype(np.float64)
        cvec = bv_tiled @ Wo[g * 512 : (g + 1) * 512, :].astype(np.float64)
        out[c // 4] += cvec.astype(np.float32)[None, :]
    return out


def kernel(x, Wq, bq, Wk, bk, Wv, bv, Wo):
    nc = _get_program()
    in_maps = make_in_maps(x, Wq, bq, Wk, bk, Wv, bv, Wo)
    res = run_bass_kernel_spmd(nc, in_maps, list(range(8)))
    return combine_outputs(res, {"bv": bv, "Wo": Wo})


# revision 13
# speedup vs baseline: 1.1558x; 1.0103x over previous
"""Trainium2 Bass kernel for causal GQA self-attention (B=2, T=2048, C=2048,
Hq=16, Hkv=4, d=128, RoPE base 1e6).

Sharding: 8 cores = 2 batches x 4 kv-head groups. Each core computes, for its
(batch b, kv group g): the q/k/v projections restricted to that group (4 q
heads + 1 kv head), RoPE, causal attention, and the partial o_proj
(y_group @ Wo[group rows]). The host sums the 4 partial o_proj outputs per
batch (the all-reduce/unshard step of tensor parallelism).

Device schedule (v2):
  - Phase 1 projections run ci-outer / t4-inner with per-t4 PSUM banks so a
    weight chunk is stationary across 4 back-to-back matmuls, and k+v are
    interleaved per-ci so PE consumption (~1.7us/ci) matches DMA arrival
    (~1.6us/ci) from the first microsecond.
  - RoPE runs fully in bf16 (cos/sin tables shipped bf16) for 2x DVE rate.
  - Attention is software-pipelined: scores for step g+1 issue before the
    AV/rowsum matmuls of step g, so the ACT exp latency hides entirely under
    PE work. Scores are computed transposed (S^T = k @ qT) so exp needs no
    cross-partition work.
  - softmax 1/rowsum: rowsums accumulate via a ones-vector matmul in PSUM;
    the reciprocal runs as one custom-DVE op straight out of PSUM, is
    broadcast across partitions by the (otherwise idle) GpSimd engine, and
    multiplies yT straight out of its PSUM bank -- no DRAM round trip, no
    ACT log/exp chain.
  - o_proj work is queued and interleaved one (ti,nj) group per attention
    step (covers per-head normalization chains); qc runs ascending so only
    the last qc's o_proj groups remain for the tail, processed with a 4-bank
    PSUM pool.
  - the v bias is folded out entirely: since softmax rows sum to 1, bv
    contributes the constant row bv_tiled @ Wo_g, added on the host.
"""

import numpy as np
import ml_dtypes

import concourse.bass as bass
import concourse.mybir as mybir
from concourse import bacc
from concourse.tile import TileContext
from concourse.bass_utils import run_bass_kernel_spmd
from concourse.masks import make_identity

BF16 = mybir.dt.bfloat16
F32 = mybir.dt.float32

T = 2048
C = 2048
D = 128
NH = 4           # q heads per core
CI = C // 128    # contraction chunks
TC = T // 512    # t chunks of 512
TB = T // 128    # t blocks of 128
SCALE = 1.0 / np.sqrt(D)

_PROGRAM = None


def _ts(i, s):
    return bass.ts(i, s)


def _patch_act_tables():
    """Force every ACT function this kernel uses to resolve to the
    natural_log_exp_and_others table set, so the whole kernel needs exactly
    one ACT_TABLE_LOAD. Returns an undo callable."""
    import concourse.bacc as bacc_mod

    orig = bacc_mod.get_activation_tables
    A = mybir.ActivationFunctionType
    mine = {A.Exp, A.Ln, A.Identity, A.Copy}

    def patched(arch):
        tables = dict(orig(arch))
        for name in tables:
            if name != "natural_log_exp_and_others":
                tables[name] = set(tables[name]) - mine
        return tables

    bacc_mod.get_activation_tables = patched

    def undo():
        bacc_mod.get_activation_tables = orig

    return undo


def _build_program():
    undo = _patch_act_tables()
    try:
        return _build_program_inner()
    finally:
        undo()


def _build_program_inner():
    nc = bacc.Bacc("TRN2", target_bir_lowering=False, debug=False, num_devices=8)

    xT_d = nc.dram_tensor("xT", [C, T], BF16, kind="ExternalInput").ap()
    wq_d = nc.dram_tensor("wq", [C, NH * D], BF16, kind="ExternalInput").ap()
    wkv_d = nc.dram_tensor("wkv", [C, 2 * D], BF16, kind="ExternalInput").ap()
    wo_d = nc.dram_tensor("wo", [NH * D, C], BF16, kind="ExternalInput").ap()
    bq_d = nc.dram_tensor("bq", [D, NH], F32, kind="ExternalInput").ap()
    bk_d = nc.dram_tensor("bk", [D, 1], F32, kind="ExternalInput").ap()
    cos_d = nc.dram_tensor("cosT", [D, T], BF16, kind="ExternalInput").ap()
    sin_d = nc.dram_tensor("sinT", [D, T], BF16, kind="ExternalInput").ap()
    tri_d = nc.dram_tensor("tri", [D, D], BF16, kind="ExternalInput").ap()
    out_d = nc.dram_tensor("out", [T, C], F32, kind="ExternalOutput").ap()

    Ident = mybir.ActivationFunctionType.Identity
    Exp = mybir.ActivationFunctionType.Exp

    with TileContext(nc) as tc:
        with (
            tc.tile_pool(name="consts", bufs=1) as consts,
            tc.tile_pool(name="acts", bufs=1) as acts,
        ):
            # ---- resident constants, DMAs issued in consumption order ----
            xT_sb = consts.tile([128, CI, T], BF16)
            wq_sb = consts.tile([128, CI, NH * D], BF16)
            wkv_sb = consts.tile([128, CI, 2 * D], BF16)
            wo_sb = consts.tile([128, NH, C], BF16)
            bq_sb = consts.tile([128, NH], F32)
            bk_sb = consts.tile([128, 1], F32)
            cos_sb = consts.tile([128, T], BF16)
            sin_sb = consts.tile([128, T], BF16)
            tri_sb = consts.tile([128, 128], BF16)
            ones_sb = consts.tile([128, 1], BF16)
            ident_sb = consts.tile([128, 128], BF16)

            nc.sync.dma_start(out=bk_sb[:], in_=bk_d[:])
            nc.sync.dma_start(out=bq_sb[:], in_=bq_d[:])
            for ci in range(CI):
                nc.sync.dma_start(out=wkv_sb[:, ci, :], in_=wkv_d[_ts(ci, 128), :])
                nc.sync.dma_start(out=xT_sb[:, ci, :], in_=xT_d[_ts(ci, 128), :])
            for ci in range(CI):
                nc.sync.dma_start(out=wq_sb[:, ci, :], in_=wq_d[_ts(ci, 128), :])
            nc.sync.dma_start(out=cos_sb[:], in_=cos_d[:])
            nc.sync.dma_start(out=sin_sb[:], in_=sin_d[:])
            nc.sync.dma_start(out=tri_sb[:], in_=tri_d[:])
            for h in range(NH):
                nc.sync.dma_start(out=wo_sb[:, h, :], in_=wo_d[_ts(h, 128), :])
            nc.vector.memset(ones_sb[:], 1.0)
            make_identity(nc, ident_sb[:])

            # ---- persistent activations ---------------------------------
            qT_all = acts.tile([128, NH, T], BF16)   # rotated q^T per head
            kT_all = acts.tile([128, T], BF16)       # rotated k^T
            v_sb = acts.tile([128, TB, D], BF16)     # v in natural [t, d] blocks
            yTn = acts.tile([128, NH, T], BF16)      # normalized y^T per head

            def rope_evict(ps_list, bias_ap, pool, tagp):
                # evict all chunks first: the 4th IDENT is what releases the
                # PSUM pool for the next phase, so do not interleave swaps
                qbs = []
                for t4, ps in enumerate(ps_list):
                    qb = qbp.tile([128, 512], BF16, name=f"{tagp}qb{t4}", tag=f"{tagp}qb{t4}")
                    nc.scalar.activation(qb[:], ps[:], Ident, bias=bias_ap)
                    qbs.append(qb)
                return qbs

            def rope_rotate(qbs, dest_of, pool):
                # rotate-half as two ACT half-copies (partition swap, sign
                # baked into the sin table), then cos/sin muls on DVE
                for t4, qb in enumerate(qbs):
                    sh = pool.tile([128, 512], BF16, tag="sh")
                    nc.scalar.copy(sh[0:64, :], qb[64:128, :])
                    nc.scalar.copy(sh[64:128, :], qb[0:64, :])
                    t1 = pool.tile([128, 512], BF16, tag="t1")
                    nc.vector.tensor_mul(t1[:], qb[:], cos_sb[:, _ts(t4, 512)])
                    nc.vector.tensor_mul(sh[:], sh[:], sin_sb[:, _ts(t4, 512)])
                    nc.vector.tensor_add(dest_of(t4), t1[:], sh[:])

            # ---- phase 1: projections ----------------------------------
            # k+v interleaved per-ci (8 PSUM banks; PE ~1.7us/ci matches the
            # 2-dispatch DMA stream), then v transposes, then q0..q3.
            with (
                tc.tile_pool(name="rope", bufs=4) as rope_pool,
                tc.tile_pool(name="qbp", bufs=1) as qbp,
            ):
                with (
                    tc.tile_pool(name="kp", bufs=1, space="PSUM") as kp,
                    tc.tile_pool(name="vp", bufs=1, space="PSUM") as vp,
                ):
                    k_ps = [kp.tile([128, 512], F32, name=f"kps{t4}", tag=f"k{t4}") for t4 in range(TC)]
                    v_ps = [vp.tile([128, 512], F32, name=f"vps{t4}", tag=f"v{t4}") for t4 in range(TC)]
                    for ci in range(CI):
                        for t4 in range(TC):
                            nc.tensor.matmul(
                                k_ps[t4][:],
                                wkv_sb[:, ci, 0:128],
                                xT_sb[:, ci, _ts(t4, 512)],
                                start=(ci == 0),
                                stop=(ci == CI - 1),
                            )
                        for t4 in range(TC):
                            nc.tensor.matmul(
                                v_ps[t4][:],
                                wkv_sb[:, ci, 128:256],
                                xT_sb[:, ci, _ts(t4, 512)],
                                start=(ci == 0),
                                stop=(ci == CI - 1),
                            )
                    vbb = []
                    for t4 in range(TC):
                        vb = qbp.tile([128, 512], BF16, name=f"vb{t4}", tag=f"vb{t4}")
                        nc.vector.tensor_copy(vb[:], v_ps[t4][:])
                        vbb.append(vb)
                    k_qbs = rope_evict(k_ps, bk_sb[:, 0:1], rope_pool, "k")
                    rope_rotate(
                        k_qbs, lambda t4: kT_all[:, _ts(t4, 512)], rope_pool
                    )

                with tc.tile_pool(name="vtp", bufs=2, space="PSUM") as vtp:
                    for t4 in range(TC):
                        for j in range(4):
                            tb = t4 * 4 + j
                            pt = vtp.tile([128, 128], BF16)
                            nc.tensor.transpose(
                                pt[:], vbb[t4][:, _ts(j, 128)], ident_sb[:]
                            )
                            nc.vector.tensor_copy(v_sb[:, tb, :], pt[:])

                with tc.tile_pool(name="qp", bufs=2, space="PSUM") as qp:
                    for h in range(NH):
                        q_ps = [qp.tile([128, 512], F32, name=f"qps{t4}", tag=f"q{t4}") for t4 in range(TC)]
                        for ci in range(CI):
                            for t4 in range(TC):
                                nc.tensor.matmul(
                                    q_ps[t4][:],
                                    wq_sb[:, ci, _ts(h, 128)],
                                    xT_sb[:, ci, _ts(t4, 512)],
                                    start=(ci == 0),
                                    stop=(ci == CI - 1),
                                )
                        q_qbs = rope_evict(q_ps, bq_sb[:, h : h + 1], rope_pool, f"q{h}")
                        rope_rotate(
                            q_qbs,
                            lambda t4, h=h: qT_all[:, h, _ts(t4, 512)],
                            rope_pool,
                        )

            # ---- phase 2: attention, pipelined, with o_proj interleave --
            oproj_q = []

            def oproj_group(ti, nj, psum_pool, oe_pool, evict):
                ps = psum_pool.tile([128, 512], F32)
                for h2 in range(NH):
                    nc.tensor.matmul(
                        ps[:],
                        yTn[:, h2, _ts(ti, 128)],
                        wo_sb[:, h2, _ts(nj, 512)],
                        start=(h2 == 0),
                        stop=(h2 == NH - 1),
                    )
                oe = oe_pool.tile([128, 512], F32)
                if evict == "v":
                    nc.vector.tensor_copy(oe[:], ps[:])
                else:
                    nc.scalar.copy(oe[:], ps[:])
                nc.sync.dma_start(
                    out=out_d[_ts(ti, 128), _ts(nj, 512)], in_=oe[:]
                )

            with (
                tc.tile_pool(name="st", bufs=2, space="PSUM") as stp,
                tc.tile_pool(name="yt", bufs=2, space="PSUM") as ytp,
                tc.tile_pool(name="rs", bufs=1, space="PSUM") as rsp,
                tc.tile_pool(name="poi", bufs=1, space="PSUM") as poi,
                tc.tile_pool(name="ptp", bufs=3) as ptp,
                tc.tile_pool(name="rsb", bufs=2) as rsb,
                tc.tile_pool(name="bcb", bufs=2) as bcb,
                tc.tile_pool(name="oei", bufs=3) as oei,
            ):
                def scores_step(h, qc, g):
                    # scores for kb pair (2g, 2g+1) + exp
                    st = stp.tile([128, 1024], F32)
                    for u in range(2):
                        nc.tensor.matmul(
                            st[:, _ts(u, 512)],
                            kT_all[:, _ts(2 * g + u, 128)],
                            qT_all[:, h, _ts(qc, 512)],
                            start=True,
                            stop=True,
                        )
                    pt = ptp.tile([128, 1024], BF16)
                    nc.scalar.activation(pt[:], st[:], Exp, scale=SCALE)
                    return pt

                for qc in range(TC):  # ascending: smallest o_proj tail
                    nkb = 4 * (qc + 1)
                    G = nkb // 2
                    for h in range(NH):
                        yt_ps = ytp.tile([128, 512], F32)
                        rs_ps = rsp.tile([1, 512], F32)
                        pt = scores_step(h, qc, 0)
                        for g in range(G):
                            # pipeline: next step's scores + exp issue first,
                            # so PE has work while ACT computes exp(g)
                            pt_next = (
                                scores_step(h, qc, g + 1) if g + 1 < G else None
                            )
                            # causal mask on diagonal blocks
                            for u in range(2):
                                kb = 2 * g + u
                                j = kb - 4 * qc
                                if j >= 0:
                                    base = u * 512
                                    if j > 0:
                                        nc.vector.memset(
                                            pt[:, base : base + j * 128], 0.0
                                        )
                                    blk = pt[:, base + j * 128 : base + (j + 1) * 128]
                                    nc.vector.tensor_mul(blk, blk, tri_sb[:])
                            for u in range(2):
                                kb = 2 * g + u
                                nc.tensor.matmul(
                                    yt_ps[:],
                                    v_sb[:, kb, :],
                                    pt[:, _ts(u, 512)],
                                    start=(kb == 0),
                                    stop=(kb == nkb - 1),
                                )
                            for u in range(2):
                                kb = 2 * g + u
                                nc.tensor.matmul(
                                    rs_ps[:],
                                    ones_sb[:],
                                    pt[:, _ts(u, 512)],
                                    start=(kb == 0),
                                    stop=(kb == nkb - 1),
                                )
                            if oproj_q and h > 0:
                                oproj_group(*oproj_q.pop(0), poi, oei, "s")
                            pt = pt_next
                        # head tail: evict yt/rs to SBUF immediately (the
                        # copies depend only on the matmul stops, so the PSUM
                        # banks rotate without waiting on the 1/s chain), then
                        # reciprocal + GpSimd partition-broadcast + normalize
                        ytu = bcb.tile([128, 512], BF16, tag="ytu")
                        nc.vector.tensor_copy(ytu[:], yt_ps[:])
                        rsc = rsb.tile([1, 512], F32, tag="rsc")
                        nc.vector.tensor_copy(rsc[:], rs_ps[:])
                        rsv = rsb.tile([1, 512], F32, tag="rsv")
                        nc.vector.reciprocal_approx_fast(rsv[:], rsc[:])
                        bc = bcb.tile([128, 512], F32, tag="bc")
                        nc.gpsimd.partition_broadcast(bc[:], rsv[:], channels=128)
                        nc.vector.tensor_mul(
                            yTn[:, h, _ts(qc, 512)], ytu[:], bc[:]
                        )
                    oproj_q.extend(
                        (ti, nj)
                        for ti in range(4 * qc, 4 * qc + 4)
                        for nj in range(TC)
                    )

            # ---- tail: drain remaining o_proj with a wide PSUM pool -----
            with (
                tc.tile_pool(name="pot", bufs=4, space="PSUM") as pot,
                tc.tile_pool(name="oet", bufs=6) as oet,
            ):
                for i, (ti, nj) in enumerate(oproj_q):
                    oproj_group(ti, nj, pot, oet, "v" if i % 2 == 0 else "s")

    nc.finalize()
    return nc


def _get_program():
    global _PROGRAM
    if _PROGRAM is None:
        _PROGRAM = _build_program()
    return _PROGRAM


def _rope_tables():
    inv_freq = 1.0 / (1000000.0 ** (np.arange(0, D, 2, dtype=np.float64) / D))
    pos = np.arange(T, dtype=np.float64)
    si = np.outer(pos, inv_freq)                      # [T, D/2]
    cos_h, sin_h = np.cos(si), np.sin(si)
    cos = np.stack([cos_h, cos_h], axis=-1).reshape(T, D)
    sin = np.stack([sin_h, sin_h], axis=-1).reshape(T, D)
    cosT = np.ascontiguousarray(cos.T).astype(np.float32)   # [D, T]
    sinT = np.ascontiguousarray(sin.T).astype(np.float32)
    # rotate-half as a partition shift: sh[i<64]=q[i+64], sh[i>=64]=q[i-64];
    # q_rot = q*cos + sh*sin_signed with the -1 for i<64 baked into the table
    sinT[: D // 2] *= -1.0
    return cosT, sinT


def make_in_maps(x, Wq, bq, Wk, bk, Wv, bv, Wo):
    bf = ml_dtypes.bfloat16
    cosT, sinT = _rope_tables()
    tri = np.triu(np.ones((D, D), dtype=np.float32)).astype(bf)  # [k, q]: q >= k
    in_maps = []
    for b in range(2):
        xT = np.ascontiguousarray(x[b].T).astype(bf)
        for g in range(4):
            in_maps.append(
                {
                    "xT": xT,
                    "wq": np.ascontiguousarray(Wq[:, g * 512 : (g + 1) * 512]).astype(bf),
                    "wkv": np.ascontiguousarray(
                        np.concatenate(
                            [
                                Wk[:, g * 128 : (g + 1) * 128],
                                Wv[:, g * 128 : (g + 1) * 128],
                            ],
                            axis=1,
                        )
                    ).astype(bf),
                    "wo": np.ascontiguousarray(Wo[g * 512 : (g + 1) * 512, :]).astype(bf),
                    "bq": np.ascontiguousarray(
                        bq[g * 512 : (g + 1) * 512].reshape(NH, D).T
                    ).astype(np.float32),
                    "bk": np.ascontiguousarray(
                        bk[g * 128 : (g + 1) * 128].reshape(D, 1)
                    ).astype(np.float32),
                    "cosT": cosT.astype(bf),
                    "sinT": sinT.astype(bf),
                    "tri": tri,
                }
            )
    return in_maps


def combine_outputs(res, inputs):
    bv, Wo = np.asarray(inputs["bv"]), np.asarray(inputs["Wo"])
    out = np.zeros((2, T, C), dtype=np.float32)
    for c in range(8):
        g = c % 4
        out[c // 4] += res.results[c]["out"]
        # v-bias contribution: softmax rows sum to 1, so bv adds the constant
        # row (bv tiled over the 4 q heads) @ Wo_group to every output row
        bv_tiled = np.tile(bv[g * 128 : (g + 1) * 128], NH).astype(np.float64)
        cvec = bv_tiled @ Wo[g * 512 : (g + 1) * 512, :].astype(np.float64)
        out[c // 4] += cvec.astype(np.float32)[None, :]
    return out


def kernel(x, Wq, bq, Wk, bk, Wv, bv, Wo):
    nc = _get_program()
    in_maps = make_in_maps(x, Wq, bq, Wk, bk, Wv, bv, Wo)
    res = run_bass_kernel_spmd(nc, in_maps, list(range(8)))
    return combine_outputs(res, {"bv": bv, "Wo": Wo})
